# revision 1
# baseline (speedup 1.0000x reference)
"""Trainium2 Bass kernel for nn_Attention_49641232007688 (sparse_attention).

Data-parallel over batch B=8 across 8 NeuronCores (one batch element per
core). Per core, everything runs in fp16 on-device (fp32 PSUM accumulation):

  1. depthwise 3x3 convs (BN-folded) on DVE: per tap a tensor_scalar mult
     (4x mode, contiguous padded stream into a temp; a host-shifted x copy
     keeps every read 4B-aligned) + tensor_tensor add (2x mode), in two
     row-groups so downstream matmuls start at half-time
  2. 1x1 convs as PE matmuls producing v, k, q in channel-major layout
     [hd, n] (A); position-major [n, hd] chunks (B) for k,v come from PE
     transposes of the A tiles, and ktv = k^T v accumulates directly in a
     long-lived PSUM bank across all 28 n-subchunks (no re-accumulation
     matmuls, no intermediate evictions)
  3. the per-head-broadcast diag (sb) comes straight from a second PE
     matmul on the qk stream with a block-diag-J64 lhsT; its evictions
     carry accum_out row-sums, so m0 = sum(sb)/64 costs no extra reduce;
     m0 crosses cores via a 1-element DRAM AllReduce
  4. oa = q@ktv - diag*v: the subtract doubles as the PSUM->SBUF eviction
     on DVE; out = Wo@oa + m0*(Wo@v) + bo with the m0 term folded in as one
     extra PSUM accumulation so the collective stays off the critical path
  5. final eviction adds bias via ACT, fp32 output streamed out per chunk;
     the back half is software-pipelined two chunks deep.

The module is built once and cached; kernel() accepts full inputs and
returns the full output.
"""

import numpy as np

HEADS = 8
DIM = 384
DIM_HEAD = 64
INNER = 512
B = 8
H = W = 56
N = H * W            # 3136
EPS = 1e-5
SCALE = DIM_HEAD ** -0.5
PC = 58              # padded cols/rows
XL = PC * PC + 4     # padded x flat length (+4 OOB slack for shifted reads)
YL = H * PC          # conv output flat length (56 rows x 58 cols)
NCH = 7              # n-chunks of 448 for 448-wide matmuls
CHW = 448
NSUB = 4             # 112-col sub-chunks per chunk (B-layout transposes)
SUBW = 112
CP = 3               # channel partition tiles (384 = 3*128)
HP = 4               # head-dim partition tiles (512 = 4*128)
RSPLIT = 56          # conv entirely on DVE (GPSIMD lacks TensorScalarPtr;
                     # a DVE-mult/Pool-add hybrid pace-couples the queues)

_CACHE = {}


def _f16(a):
    return np.ascontiguousarray(a, dtype=np.float16)


def _build(reps: int = 1, loop_n=None, no_cc=False):
    import concourse.bacc as bacc
    import concourse.mybir as mybir
    import concourse.tile as tile

    F16 = mybir.dt.float16
    F32 = mybir.dt.float32
    ALU = mybir.AluOpType
    ACTF = mybir.ActivationFunctionType

    nc = bacc.Bacc(None, num_devices=8)

    # ---- DRAM I/O ----
    xp_d = nc.dram_tensor("xp", [DIM, XL], F16, kind="ExternalInput")
    xp1_d = nc.dram_tensor("xp1", [DIM, XL], F16, kind="ExternalInput")
    tq_d = nc.dram_tensor("tq", [DIM, 9], F32, kind="ExternalInput")
    bq_d = nc.dram_tensor("bq", [DIM, 1], F32, kind="ExternalInput")
    tk_d = nc.dram_tensor("tk", [DIM, 9], F32, kind="ExternalInput")
    bk_d = nc.dram_tensor("bk", [DIM, 1], F32, kind="ExternalInput")
    wqT_d = nc.dram_tensor("wqT", [DIM, INNER], F16, kind="ExternalInput")
    wkvT_d = nc.dram_tensor("wkvT", [DIM, 2 * INNER], F16, kind="ExternalInput")
    woT_d = nc.dram_tensor("woT", [INNER, DIM], F16, kind="ExternalInput")
    bo_d = nc.dram_tensor("bo", [DIM, 1], F32, kind="ExternalInput")
    hsel2_d = nc.dram_tensor("hsel2", [128, 128], F16, kind="ExternalInput")
    o64_d = nc.dram_tensor("o64", [128, 1], F32, kind="ExternalInput")
    posI_d = nc.dram_tensor("posI", [128, 128], F16, kind="ExternalInput")
    out_d = nc.dram_tensor("out", [DIM, N], F32, kind="ExternalOutput")

    def ch_cols(t, ch):
        return t[:, ch * CHW:(ch + 1) * CHW]

    with tile.TileContext(nc) as tc:
        with (
            tc.tile_pool(name="wsb", bufs=1) as wsb,
            tc.tile_pool(name="xsb", bufs=1) as xsb,
            tc.tile_pool(name="ysb", bufs=1) as ysb,
            tc.tile_pool(name="big", bufs=1) as big,
            tc.tile_pool(name="small", bufs=2) as small,
            tc.tile_pool(name="ev", bufs=3) as ev,
            tc.tile_pool(name="mm_ps", bufs=3, space="PSUM") as mm_ps,
            tc.tile_pool(name="kv_ps", bufs=2, space="PSUM") as kv_ps,
            tc.tile_pool(name="ktv_ps", bufs=1, space="PSUM") as ktv_ps,
            tc.tile_pool(name="dps", bufs=1, space="PSUM") as dps_pool,
            tc.tile_pool(name="dram", bufs=1, space="DRAM") as dram,
        ):
            def emit_body():
                # ---- load padded x (first: the convs gate everything) ----
                x0_t = [xsb.tile([128, XL], F16, tag=f"x0{p}", name=f"x0{p}") for p in range(CP)]
                x1_t = [xsb.tile([128, XL], F16, tag=f"x1{p}", name=f"x1{p}") for p in range(CP)]
                G0E = 26 * PC
                G1S = 24 * PC
                # DMA-queue order drives the ramp: the tiny conv tap/bias
                # scalars go FIRST (the convs cannot start without them),
                # then x group 1, then wkv (gates loop A), then x group 2,
                # then everything else.

                # ---- load weights ----
                wq_t = [wsb.tile([128, INNER], F16, tag=f"wq{p}", name=f"wq{p}") for p in range(CP)]
                wkv_t = [wsb.tile([128, 2 * INNER], F16, tag=f"wkv{p}", name=f"wkv{p}") for p in range(CP)]
                wo_t = [wsb.tile([128, DIM], F16, tag=f"wo{p}", name=f"wo{p}") for p in range(HP)]
                tq_t = [wsb.tile([128, 9], F32, tag=f"tq{p}", name=f"tq{p}") for p in range(CP)]
                bq_t = [wsb.tile([128, 1], F32, tag=f"bq{p}", name=f"bq{p}") for p in range(CP)]
                tk_t = [wsb.tile([128, 9], F32, tag=f"tk{p}", name=f"tk{p}") for p in range(CP)]
                bk_t = [wsb.tile([128, 1], F32, tag=f"bk{p}", name=f"bk{p}") for p in range(CP)]
                bo_t = [wsb.tile([128, 1], F32, tag=f"bo{p}", name=f"bo{p}") for p in range(CP)]
                hsel2_t = wsb.tile([128, 128], F16, tag="hsel2", name="hsel2")
                o64_t = wsb.tile([128, 1], F32, tag="o64", name="o64")
                posI_t = wsb.tile([128, 128], F16, tag="posI", name="posI")
                for p in range(CP):
                    cs = slice(p * 128, (p + 1) * 128)
                    nc.sync.dma_start(out=tk_t[p], in_=tk_d[cs, :])
                    nc.sync.dma_start(out=bk_t[p], in_=bk_d[cs, :])
                    nc.sync.dma_start(out=tq_t[p], in_=tq_d[cs, :])
                    nc.sync.dma_start(out=bq_t[p], in_=bq_d[cs, :])
                for p in range(CP):
                    cs = slice(p * 128, (p + 1) * 128)
                    nc.sync.dma_start(out=x0_t[p][:, 0:G0E], in_=xp_d[cs, 0:G0E])
                    nc.sync.dma_start(out=x1_t[p][:, 0:G0E], in_=xp1_d[cs, 0:G0E])
                for p in range(CP):
                    cs = slice(p * 128, (p + 1) * 128)
                    nc.sync.dma_start(out=wkv_t[p], in_=wkvT_d[cs, :])
                for p in range(CP):
                    cs = slice(p * 128, (p + 1) * 128)
                    nc.sync.dma_start(out=x0_t[p][:, G1S:XL], in_=xp_d[cs, G1S:XL])
                    nc.sync.dma_start(out=x1_t[p][:, G1S:XL], in_=xp1_d[cs, G1S:XL])
                nc.sync.dma_start(out=posI_t, in_=posI_d[:, :])
                for p in range(CP):
                    cs = slice(p * 128, (p + 1) * 128)
                    nc.sync.dma_start(out=wq_t[p], in_=wqT_d[cs, :])
                    nc.sync.dma_start(out=bo_t[p], in_=bo_d[cs, :])
                for p in range(HP):
                    cs = slice(p * 128, (p + 1) * 128)
                    nc.sync.dma_start(out=wo_t[p], in_=woT_d[cs, :])
                nc.sync.dma_start(out=hsel2_t, in_=hsel2_d[:, :])
                nc.sync.dma_start(out=o64_t, in_=o64_d[:, :])

                # ---- depthwise convs: rows [0,RSPLIT) DVE / [RSPLIT,H) GPSIMD --
                # DVE: tensor_scalar mul (4x mode, contiguous padded stream
                # into a temp; host-shifted x copy keeps dx==1 4B-aligned) +
                # tensor_tensor add (2x mode). GPSIMD: fused stt MACs (no
                # perf modes there, one op per tap).
                RG = ((0, 24), (24, H))

                def conv_dve(ys, taps, bias, r0, r1):
                    rows = r1 - r0
                    ve = nc.vector
                    for p in range(CP):
                        y = ys[p]
                        x0v = x0_t[p][:, 0:PC * PC].rearrange("p (a b) -> p a b", b=PC)
                        x1v = x1_t[p][:, 0:PC * PC].rearrange("p (a b) -> p a b", b=PC)
                        first = True
                        for dy in range(3):
                            for dx in range(3):
                                i = dy * 3 + dx
                                if dx == 1:
                                    src = x1v[:, r0 + dy:r1 + dy, 0:W]
                                else:
                                    src = x0v[:, r0 + dy:r1 + dy, dx:dx + W]
                                if first:
                                    ve.tensor_scalar(
                                        out=y[:, r0:r1, :], in0=src,
                                        scalar1=taps[p][:, i:i + 1],
                                        scalar2=bias[p],
                                        op0=ALU.mult, op1=ALU.add)
                                    first = False
                                else:
                                    off = (r0 + dy) * PC + dx
                                    glen = rows * PC
                                    t = ysb.tile([128, glen], F16, tag="tconv",
                                                 name="tconv", bufs=2,
                                                 padded_shape=[128, 32 * PC + 4])
                                    if off % 2 == 0:
                                        tsrc = x0_t[p][:, off:off + glen]
                                    else:
                                        tsrc = x1_t[p][:, off - 1:off - 1 + glen]
                                    ve.tensor_scalar(
                                        out=t, in0=tsrc,
                                        scalar1=taps[p][:, i:i + 1],
                                        scalar2=None, op0=ALU.mult)
                                    tv = t.rearrange("p (a b) -> p a b", b=PC)
                                    ve.tensor_tensor(
                                        out=y[:, r0:r1, :], in0=tv[:, 0:rows, 0:W],
                                        in1=y[:, r0:r1, :], op=ALU.add)

                yq_t = [ysb.tile([128, H, W], F16, tag=f"yq{p}", name=f"yq{p}")
                        for p in range(CP)]
                ykv_t = [ysb.tile([128, H, W], F16, tag=f"ykv{p}", name=f"ykv{p}")
                         for p in range(CP)]
                y_kv = [y.rearrange("p a b -> p (a b)") for y in ykv_t]
                y_q = [y.rearrange("p a b -> p (a b)") for y in yq_t]

                def y_chunk(y, ch):
                    return y[:, ch * CHW:(ch + 1) * CHW]

                # ---- persistent layout-A tiles ----
                vA = [big.tile([128, N], F16, tag=f"vA{m}", name=f"vA{m}") for m in range(HP)]
                kA = [big.tile([128, N], F16, tag=f"kA{m}", name=f"kA{m}") for m in range(HP)]
                qA = [big.tile([128, N], F16, tag=f"qA{m}", name=f"qA{m}") for m in range(HP)]

                # ---- front loop A: v,k projections + B-layout transposes +
                # direct PSUM ktv accumulation ----
                ktv_acc = ktv_ps.tile([128, HP, 128], F32, tag="ktva", name="ktva")

                def emit_A(ch):
                    for m in range(HP):
                        ps = mm_ps.tile([128, CHW], F32, tag="mm", name="mm")
                        for p in range(CP):
                            nc.tensor.matmul(
                                out=ps[:, :],
                                lhsT=wkv_t[p][:, INNER + m * 128:INNER + (m + 1) * 128],
                                rhs=y_chunk(y_kv[p], ch),
                                start=(p == 0), stop=(p == CP - 1))
                        nc.scalar.copy(out=ch_cols(vA[m], ch), in_=ps[:, :])
                    for m in range(HP):
                        ps = mm_ps.tile([128, CHW], F32, tag="mm", name="mm")
                        for p in range(CP):
                            nc.tensor.matmul(
                                out=ps[:, :],
                                lhsT=wkv_t[p][:, m * 128:(m + 1) * 128],
                                rhs=y_chunk(y_kv[p], ch),
                                start=(p == 0), stop=(p == CP - 1))
                        nc.scalar.copy(out=ch_cols(kA[m], ch), in_=ps[:, :])
                    for sub in range(NSUB):
                        cols = slice(ch * CHW + sub * SUBW, ch * CHW + (sub + 1) * SUBW)
                        kvb_ps = kv_ps.tile([SUBW, 2 * INNER], F16, tag="kvb", name="kvb_ps")
                        for m in range(HP):
                            nc.tensor.transpose(
                                out=kvb_ps[:, m * 128:(m + 1) * 128],
                                in_=kA[m][:, cols], identity=posI_t)
                        for m in range(HP):
                            nc.tensor.transpose(
                                out=kvb_ps[:, INNER + m * 128:INNER + (m + 1) * 128],
                                in_=vA[m][:, cols], identity=posI_t)
                        kvb16 = ev.tile([SUBW, 2 * INNER], F16, tag="kvb16",
                                        name="kvb16", bufs=2)
                        nc.scalar.copy(out=kvb16, in_=kvb_ps[:, :])
                        for mp in range(HP):
                            ms = slice(mp * 128, (mp + 1) * 128)
                            # start zeroes the whole 2KB bank, so only the
                            # very first matmul of the 4 interleaved chains
                            # may set it (and only the very last sets stop)
                            nc.tensor.matmul(
                                out=ktv_acc[:, mp, :],
                                lhsT=kvb16[:, ms],
                                rhs=kvb16[:, INNER + mp * 128:INNER + (mp + 1) * 128],
                                start=(ch == 0 and sub == 0 and mp == 0),
                                stop=(ch == NCH - 1 and sub == NSUB - 1 and mp == HP - 1))

                # ---- front loop B: q projection + diag broadcast ----
                # Per chunk: stream the 4 m-tiles' q projections on PE with
                # no interleaved dependents (qk reads the PSUM directly on
                # DVE; qA eviction on ACT runs in parallel), then emit the
                # previous chunk's sb matmuls (lhsT = block-diag J64 turns
                # the qk stream directly into per-head-broadcast diag).
                sb16 = [xsb.tile([128, N], F16, tag=f"x0{m}" if m < CP else "x10",
                                 name=f"sb{m}") for m in range(HP)]
                # per-(m, chunk) row-sum partials of sb (free via accum_out on
                # the evictions) -> m0 without any standalone big reduce
                parts = [small.tile([128, 8], F32, tag=f"part{m}", name=f"part{m}",
                                    bufs=1) for m in range(HP)]
                for m in range(HP):
                    nc.vector.memset(parts[m], 0.0)

                def emit_sb(ch, qks):
                    for m in range(HP):
                        sb_ps = kv_ps.tile([128, CHW], F32, tag="sbps",
                                           name="sbps")
                        nc.tensor.matmul(out=sb_ps[:, :], lhsT=hsel2_t,
                                         rhs=qks[m], start=True, stop=True)
                        if m % 2 == 0:
                            # op1 doubles as the accum_out reduce op
                            nc.vector.tensor_scalar(
                                out=ch_cols(sb16[m], ch), in0=sb_ps[:, :],
                                scalar1=1.0, scalar2=0.0, op0=ALU.mult,
                                op1=ALU.add,
                                accum_out=parts[m][:, ch:ch + 1])
                        else:
                            nc.scalar.activation(
                                out=ch_cols(sb16[m], ch), in_=sb_ps[:, :],
                                func=ACTF.Copy,
                                accum_out=parts[m][:, ch:ch + 1])

                _prev_qk = [None]

                def emit_B(ch):
                    prev_qk = _prev_qk[0]
                    cur_qk = []
                    for m in range(HP):
                        ps = mm_ps.tile([128, CHW], F32, tag="mm", name="mm")
                        for p in range(CP):
                            nc.tensor.matmul(
                                out=ps[:, :],
                                lhsT=wq_t[p][:, m * 128:(m + 1) * 128],
                                rhs=y_chunk(y_q[p], ch),
                                start=(p == 0), stop=(p == CP - 1))
                        nc.scalar.copy(out=ch_cols(qA[m], ch), in_=ps[:, :])
                        qk = ev.tile([128, CHW], F16, tag=f"qk{m}",
                                     name=f"qk{m}", bufs=2)
                        nc.vector.tensor_tensor(out=qk, in0=ch_cols(qA[m], ch),
                                                in1=ch_cols(kA[m], ch), op=ALU.mult)
                        cur_qk.append(qk)
                    if prev_qk is not None:
                        emit_sb(ch - 1, prev_qk)
                    _prev_qk[0] = cur_qk

                conv_dve(ykv_t, tk_t, bk_t, *RG[0])
                conv_dve(ykv_t, tk_t, bk_t, *RG[1])
                conv_dve(yq_t, tq_t, bq_t, *RG[0])
                conv_dve(yq_t, tq_t, bq_t, *RG[1])
                for ch in range(NCH):
                    emit_A(ch)
                for ch in range(NCH):
                    emit_B(ch)
                emit_sb(NCH - 1, _prev_qk[0])

                # ---- m0: global scalar via 1-element AllReduce ----
                # m0 = sum(diag) = sum(sb16)/64 (every head row repeats 64x)
                m0_ps = kv_ps.tile([1, 8], F32, tag="sbps", name="m0ps")
                for m in range(HP):
                    nc.tensor.matmul(out=m0_ps[:, :], lhsT=o64_t, rhs=parts[m],
                                     start=(m == 0), stop=(m == HP - 1))
                m0s = small.tile([1, 1], F32, tag="m0s", name="m0s")
                nc.vector.tensor_reduce(out=m0s, in_=m0_ps[:, :],
                                        axis=mybir.AxisListType.X, op=ALU.add)
                cc = dram.tile([1, 1], F32, tag="cc", name="cc")
                nc.gpsimd.dma_start(out=cc[:, :], in_=m0s)
                if not no_cc:
                    nc.gpsimd.collective_compute(
                        "AllReduce", ALU.add, replica_groups=[list(range(8))],
                        ins=[cc[:, :].opt()], outs=[cc[:, :].opt()])
                m0b = small.tile([128, 1], F32, tag="m0b", name="m0b")
                nc.gpsimd.dma_start(out=m0b, in_=cc[:, :].partition_broadcast(128))
                # m0I = m0 * I -- folds the m0*v term into the final PSUM
                # accumulation, so the collective overlaps the whole back half.
                m0I = small.tile([128, 128], F16, tag="m0I", name="m0I")
                nc.gpsimd.tensor_scalar(out=m0I, in0=posI_t, scalar1=m0b,
                                        scalar2=None, op0=ALU.mult)

                # ---- ktv block-diagonal per m-tile ----
                ktv_bd = small.tile([128, HP, 128], F16, tag="ktvbd", name="ktvbd")
                nc.vector.memset(ktv_bd, 0.0)
                nc.scalar.copy(out=ktv_bd[0:DIM_HEAD, :, 0:DIM_HEAD],
                               in_=ktv_acc[0:DIM_HEAD, :, 0:DIM_HEAD])
                nc.scalar.copy(out=ktv_bd[DIM_HEAD:128, :, DIM_HEAD:128],
                               in_=ktv_acc[DIM_HEAD:128, :, DIM_HEAD:128])

                # ---- back half: oa + final matmul, streamed per chunk ----
                # software-pipelined: fps(ch-1) is emitted between wov(ch)
                # and oa(ch) so PE never sits behind the DVE oa evictions.
                def emit_wov(ch):
                    wov16 = []
                    for ot in range(CP):
                        ps = mm_ps.tile([128, CHW], F32, tag="mm", name="mm")
                        for kt in range(HP):
                            nc.tensor.matmul(
                                out=ps[:, :],
                                lhsT=wo_t[kt][:, ot * 128:(ot + 1) * 128],
                                rhs=ch_cols(vA[kt], ch),
                                start=(kt == 0), stop=(kt == HP - 1))
                        w16 = ev.tile([128, CHW], F16, tag=f"wov{ot}",
                                      name=f"wov{ot}", bufs=3)
                        nc.scalar.copy(out=w16, in_=ps[:, :])
                        wov16.append(w16)
                    return wov16

                def emit_oa(ch):
                    oach = []
                    for m in range(HP):
                        ew = ev.tile([128, CHW], F16, tag="ew", name="ew", bufs=2)
                        nc.vector.tensor_tensor(out=ew, in0=ch_cols(sb16[m], ch),
                                                in1=ch_cols(vA[m], ch), op=ALU.mult)
                        oa_ps = mm_ps.tile([128, CHW], F32, tag="mm", name="mm")
                        nc.tensor.matmul(out=oa_ps[:, :], lhsT=ktv_bd[:, m, :],
                                         rhs=ch_cols(qA[m], ch),
                                         start=True, stop=True)
                        oa = ev.tile([128, CHW], F16, tag=f"oa{m}", name=f"oa{m}",
                                     bufs=3)
                        nc.vector.tensor_tensor(out=oa, in0=oa_ps[:, :], in1=ew,
                                                op=ALU.subtract)
                        oach.append(oa)
                    return oach

                def emit_final(ch, wov16, oach):
                    for ot in range(CP):
                        ps = kv_ps.tile([128, CHW], F32, tag="sbps", name="fps")
                        for kt in range(HP):
                            nc.tensor.matmul(
                                out=ps[:, :],
                                lhsT=wo_t[kt][:, ot * 128:(ot + 1) * 128],
                                rhs=oach[kt],
                                start=(kt == 0), stop=False)
                        nc.tensor.matmul(out=ps[:, :], lhsT=m0I,
                                         rhs=wov16[ot],
                                         start=False, stop=True)
                        of = ev.tile([128, CHW], F32, tag="of", name="of", bufs=2)
                        nc.scalar.activation(out=of, in_=ps[:, :],
                                             func=ACTF.Identity,
                                             bias=bo_t[ot], scale=1.0)
                        nc.sync.dma_start(
                            out=out_d[ot * 128:(ot + 1) * 128,
                                      ch * CHW:(ch + 1) * CHW],
                            in_=of)

                pend = []
                for ch in range(NCH):
                    wov16 = emit_wov(ch)
                    oach = emit_oa(ch)
                    pend.append((ch, wov16, oach))
                    if len(pend) > 2:
                        e = pend.pop(0)
                        emit_final(e[0], e[1], e[2])
                for e in pend:
                    emit_final(e[0], e[1], e[2])
            if loop_n is None:
                for _ in range(reps):
                    emit_body()
            else:
                with tc.For_i(0, loop_n, 1):
                    emit_body()
    nc.finalize()
    return nc


def _get_nc(reps: int = 1, loop_n=None, no_cc=False):
    key = (reps, loop_n, no_cc)
    if key not in _CACHE:
        _CACHE[key] = _build(reps, loop_n, no_cc)
    return _CACHE[key]


def prepare_in_maps(inputs):
    """Host-side preprocessing: fold BN, pad/shift x, transpose weights."""
    x = np.asarray(inputs["x"], np.float32)

    def fold(dw, g, b, m, v):
        inv = np.asarray(g, np.float32) / np.sqrt(np.asarray(v, np.float32) + EPS)
        taps = np.asarray(dw, np.float32)[:, 0].reshape(DIM, 9) * inv[:, None]
        bias = np.asarray(b, np.float32) - np.asarray(m, np.float32) * inv
        return (np.ascontiguousarray(taps, np.float32),
                np.ascontiguousarray(bias[:, None], np.float32))

    tq, bq = fold(inputs["wq_dw"], inputs["wq_bn_g"], inputs["wq_bn_b"],
                  inputs["wq_bn_m"], inputs["wq_bn_v"])
    tk, bk = fold(inputs["wkv_dw"], inputs["wkv_bn_g"], inputs["wkv_bn_b"],
                  inputs["wkv_bn_m"], inputs["wkv_bn_v"])
    wqT = _f16((SCALE * np.asarray(inputs["wq_pw"], np.float32)).T)
    wkvT = _f16(np.asarray(inputs["wkv_pw"], np.float32).T)
    woT = _f16(np.asarray(inputs["wo"], np.float32).T)
    bo = np.ascontiguousarray(np.asarray(inputs["bo"], np.float32)[:, None])
    hsel2 = np.zeros((128, 128), np.float32)
    hsel2[:64, :64] = 1.0
    hsel2[64:, 64:] = 1.0
    hsel2 = _f16(hsel2)
    posI = _f16(np.eye(128, dtype=np.float32))
    o64 = np.ascontiguousarray(np.full((128, 1), 1.0 / DIM_HEAD, np.float32))

    xpad = np.zeros((B, DIM, PC, PC), np.float16)
    xpad[:, :, 1:1 + H, 1:1 + W] = x.astype(np.float16)
    xflat = np.zeros((B, DIM, XL), np.float16)
    xflat[:, :, :PC * PC] = xpad.reshape(B, DIM, PC * PC)
    xsh = np.zeros_like(xflat)
    xsh[:, :, :XL - 1] = xflat[:, :, 1:]
    shared = dict(tq=tq, bq=bq, tk=tk, bk=bk, wqT=wqT, wkvT=wkvT, woT=woT,
                  bo=bo, hsel2=hsel2, posI=posI, o64=o64)
    return [dict(shared, xp=np.ascontiguousarray(xflat[b]),
                 xp1=np.ascontiguousarray(xsh[b])) for b in range(B)]


def kernel(**inputs) -> np.ndarray:
    from concourse.bass_utils import run_bass_kernel_spmd
    in_maps = prepare_in_maps(inputs)
    nc = _get_nc(1)
    res = run_bass_kernel_spmd(nc, in_maps, list(range(8)))
    out = np.stack([res.results[b]["out"] for b in range(B)])
    return np.ascontiguousarray(out.reshape(B, DIM, H, W).astype(np.float32))



# revision 58
# speedup vs baseline: 5.6888x; 5.6888x over previous
"""Trainium2 Bass kernel for nn_Attention_49641232007688 (sparse_attention).

Data-parallel over batch B=8 across 8 NeuronCores (one batch element per
core). Per core, everything runs in fp16 on-device (fp32 PSUM accumulation):

  1. depthwise 3x3 convs (BN-folded) on DVE: per tap a tensor_scalar mult
     (4x mode, contiguous padded stream into a temp; a host-shifted x copy
     keeps every read 4B-aligned) + tensor_tensor add (2x mode), in two
     row-groups so downstream matmuls start at half-time
  2. 1x1 convs as PE matmuls producing v, k, q in channel-major layout
     [hd, n] (A); position-major [n, hd] chunks (B) for k,v come from PE
     transposes of the A tiles, and ktv = k^T v accumulates directly in a
     long-lived PSUM bank across all 28 n-subchunks (no re-accumulation
     matmuls, no intermediate evictions)
  3. the per-head-broadcast diag (sb) comes straight from a second PE
     matmul on the qk stream with a block-diag-J64 lhsT; its evictions
     carry accum_out row-sums, so m0 = sum(sb)/64 costs no extra reduce;
     m0 crosses cores via a 1-element DRAM AllReduce
  4. oa = q@ktv - diag*v: the subtract doubles as the PSUM->SBUF eviction
     on DVE; out = Wo@oa + m0*(Wo@v) + bo with the m0 term folded in as one
     extra PSUM accumulation so the collective stays off the critical path
  5. final eviction adds bias via ACT, fp32 output streamed out per chunk;
     the back half is software-pipelined two chunks deep.

The module is built once and cached; kernel() accepts full inputs and
returns the full output.
"""

import numpy as np

HEADS = 8
DIM = 384
DIM_HEAD = 64
INNER = 512
B = 8
H = W = 56
N = H * W            # 3136
EPS = 1e-5
SCALE = DIM_HEAD ** -0.5
PC = 58              # padded cols/rows
XL = PC * PC + 4     # padded x flat length (+4 OOB slack for shifted reads)
YL = H * PC          # conv output flat length (56 rows x 58 cols)
NCH = 7              # n-chunks of 448 for 448-wide matmuls
CHW = 448
NSUB = 4             # 112-col sub-chunks per chunk (B-layout transposes)
SUBW = 112
CP = 3               # channel partition tiles (384 = 3*128)
HP = 4               # head-dim partition tiles (512 = 4*128)
DVR = 37             # conv rows [0,DVR) on DVE (4x mult + 2x add),
                     # rows [DVR,H) on Pool via fused scalar_tensor_tensor
                     # (1x but a whole tap in one op on an idle engine)

_CACHE = {}


def _f16(a):
    return np.ascontiguousarray(a, dtype=np.float16)


DEFAULT_CFG = dict(kd=41, qd=41, per0k=41, per0q=41, oa_mode="dve", pstep=8)


def _build(reps: int = 1, loop_n=None, no_cc=False, cfg=None):
    cfg = dict(DEFAULT_CFG, **(cfg or {}))
    kd, qd, oa_mode = cfg["kd"], cfg["qd"], cfg["oa_mode"]
    per0k, per0q = cfg["per0k"], cfg["per0q"]
    import concourse.bacc as bacc
    import concourse.mybir as mybir
    import concourse.tile as tile

    F16 = mybir.dt.float16
    F32 = mybir.dt.float32
    ALU = mybir.AluOpType
    ACTF = mybir.ActivationFunctionType

    nc = bacc.Bacc(None, num_devices=8)

    # ---- DRAM I/O ----
    xp_d = nc.dram_tensor("xp", [DIM, XL], F16, kind="ExternalInput")
    # packed conv scalars: [tk(9) | bk(1) | tq(9) | bq(1)] -- one DMA per
    # channel tile instead of four (each descriptor costs ~0.65us of DMA
    # queue-head latency, and these gate the convs)
    tb_d = nc.dram_tensor("tb", [DIM, 20], F32, kind="ExternalInput")
    wqT_d = nc.dram_tensor("wqT", [DIM, INNER], F16, kind="ExternalInput")
    wkvT_d = nc.dram_tensor("wkvT", [DIM, 2 * INNER], F16, kind="ExternalInput")
    woT_d = nc.dram_tensor("woT", [INNER, DIM], F16, kind="ExternalInput")
    bo_d = nc.dram_tensor("bo", [DIM, 1], F32, kind="ExternalInput")
    hsel2_d = nc.dram_tensor("hsel2", [128, 128], F16, kind="ExternalInput")
    o64_d = nc.dram_tensor("o64", [128, 1], F32, kind="ExternalInput")
    posI_d = nc.dram_tensor("posI", [128, 128], F16, kind="ExternalInput")
    dg_d = nc.dram_tensor("dg", [128, 54 * 128], F16, kind="ExternalInput")
    out_d = nc.dram_tensor("out", [DIM, N], F32, kind="ExternalOutput")

    def ch_cols(t, ch):
        return t[:, ch * CHW:(ch + 1) * CHW]

    with tile.TileContext(nc) as tc:
        with (
            tc.tile_pool(name="wsb", bufs=1) as wsb,
            tc.tile_pool(name="xsb", bufs=1) as xsb,
            tc.tile_pool(name="ysb", bufs=1) as ysb,
            tc.tile_pool(name="big", bufs=1) as big,
            tc.tile_pool(name="small", bufs=2) as small,
            tc.tile_pool(name="ev", bufs=3) as ev,
            tc.tile_pool(name="mm_ps", bufs=3, space="PSUM") as mm_ps,
            tc.tile_pool(name="kv_ps", bufs=2, space="PSUM") as kv_ps,
            tc.tile_pool(name="ktv_ps", bufs=1, space="PSUM") as ktv_ps,
            tc.tile_pool(name="dps", bufs=1, space="PSUM") as dps_pool,
            tc.tile_pool(name="dram", bufs=1, space="DRAM") as dram,
        ):
            def emit_body():
                # ---- load padded x (first: the convs gate everything) ----
                XA = 10 * PC          # rows 0-10: DVE group (0,8) + halo
                XP0 = min(kd, qd) * PC   # Pool band x0 start
                XB = 26 * PC          # rows 10-26: DVE ladder continuation
                x0_t = [xsb.tile([128, XL], F16, tag=f"x0{p}", name=f"x0{p}") for p in range(CP)]
                # DMA-queue order drives the ramp: the tiny conv tap/bias
                # scalars go FIRST (the convs cannot start without them),
                # then x rows 0-10 (DVE ladder start), then the Pool band,
                # then x rows 10-26, wkv (gates loop A), the x remainder,
                # then everything else.

                # ---- load weights ----
                wq_t = [wsb.tile([128, INNER], F16, tag=f"wq{p}", name=f"wq{p}") for p in range(CP)]
                wkv_t = [wsb.tile([128, 2 * INNER], F16, tag=f"wkv{p}", name=f"wkv{p}") for p in range(CP)]
                wo_t = [wsb.tile([128, DIM], F16, tag=f"wo{p}", name=f"wo{p}") for p in range(HP)]
                tb_t = [wsb.tile([128, 20], F32, tag=f"tb{p}", name=f"tb{p}") for p in range(CP)]
                tk_t = [t[:, 0:9] for t in tb_t]
                bk_t = [t[:, 9:10] for t in tb_t]
                tq_t = [t[:, 10:19] for t in tb_t]
                bq_t = [t[:, 19:20] for t in tb_t]
                bo_t = [wsb.tile([128, 1], F32, tag=f"bo{p}", name=f"bo{p}") for p in range(CP)]
                hsel2_t = wsb.tile([128, 128], F16, tag="hsel2", name="hsel2")
                o64_t = wsb.tile([128, 1], F32, tag="o64", name="o64")
                posI_t = wsb.tile([128, 128], F16, tag="posI", name="posI")
                for p in range(CP):
                    cs = slice(p * 128, (p + 1) * 128)
                    nc.sync.dma_start(out=tb_t[p], in_=tb_d[cs, :])
                for p in range(CP):
                    cs = slice(p * 128, (p + 1) * 128)
                    nc.sync.dma_start(out=x0_t[p][:, 0:XA], in_=xp_d[cs, 0:XA])
                for p in range(CP):
                    cs = slice(p * 128, (p + 1) * 128)
                    nc.sync.dma_start(out=x0_t[p][:, XB:XL], in_=xp_d[cs, XB:XL])
                for p in range(CP):
                    cs = slice(p * 128, (p + 1) * 128)
                    nc.sync.dma_start(out=x0_t[p][:, XA:XB], in_=xp_d[cs, XA:XB])
                for p in range(CP):
                    cs = slice(p * 128, (p + 1) * 128)
                    nc.sync.dma_start(out=wkv_t[p], in_=wkvT_d[cs, :])
                dg_k = wsb.tile([128, 27, 128], F16, tag="dg", name="dgk")
                nc.sync.dma_start(out=dg_k, in_=dg_d[:, 0:27 * 128])
                nc.sync.dma_start(out=posI_t, in_=posI_d[:, :])
                for p in range(CP):
                    cs = slice(p * 128, (p + 1) * 128)
                    nc.sync.dma_start(out=wq_t[p], in_=wqT_d[cs, :])
                    nc.sync.dma_start(out=bo_t[p], in_=bo_d[cs, :])
                for p in range(HP):
                    cs = slice(p * 128, (p + 1) * 128)
                    nc.sync.dma_start(out=wo_t[p], in_=woT_d[cs, :])
                nc.sync.dma_start(out=hsel2_t, in_=hsel2_d[:, :])
                nc.sync.dma_start(out=o64_t, in_=o64_d[:, :])

                # ---- depthwise convs: rows [0,DVR) DVE / [DVR,H) Pool ----
                # DVE: tensor_scalar mul (4x mode, contiguous padded stream
                # into a temp; host-shifted x copy keeps dx==1 4B-aligned) +
                # tensor_tensor add (2x mode). Pool: one fused
                # scalar_tensor_tensor MAC per tap straight off the strided
                # x view (1x, but a whole tap per op on an idle engine).
                # Groups are laddered so loop A's first chunks unblock early
                # and each engine's queue is ordered by consumer deadline.

                def conv_dve(ys, taps, bias, r0, r1):
                    rows = r1 - r0
                    ve = nc.vector
                    for p in range(CP):
                        y = ys[p]
                        x0v = x0_t[p][:, 0:PC * PC].rearrange("p (a b) -> p a b", b=PC)
                        first = True
                        for dy in range(3):
                            for dx in range(3):
                                i = dy * 3 + dx
                                if first:
                                    src = x0v[:, r0 + dy:r1 + dy, dx:dx + W]
                                    ve.tensor_scalar(
                                        out=y[:, r0:r1, :], in0=src,
                                        scalar1=taps[p][:, i:i + 1],
                                        scalar2=bias[p],
                                        op0=ALU.mult, op1=ALU.add)
                                    first = False
                                elif dx != 1:
                                    # contiguous 58-grid stream: 4x mode
                                    off = (r0 + dy) * PC + dx
                                    glen = rows * PC
                                    t = ysb.tile([128, glen], F16, tag="tconv",
                                                 name="tconv", bufs=2,
                                                 padded_shape=[128, 24 * PC + 4])
                                    ve.tensor_scalar(
                                        out=t, in0=x0_t[p][:, off:off + glen],
                                        scalar1=taps[p][:, i:i + 1],
                                        scalar2=None, op0=ALU.mult)
                                    tv = t.rearrange("p (a b) -> p a b", b=PC)
                                    ve.tensor_tensor(
                                        out=y[:, r0:r1, :], in0=tv[:, 0:rows, 0:W],
                                        in1=y[:, r0:r1, :], op=ALU.add)
                                else:
                                    # dx==1 is odd-offset: strided 2x mult
                                    # into a compact temp (no shifted x copy)
                                    glen = rows * W
                                    t = ysb.tile([128, glen], F16, tag="tconv",
                                                 name="tconv", bufs=2,
                                                 padded_shape=[128, 24 * PC + 4])
                                    ve.tensor_scalar(
                                        out=t, in0=x0v[:, r0 + dy:r1 + dy, 1:1 + W],
                                        scalar1=taps[p][:, i:i + 1],
                                        scalar2=None, op0=ALU.mult)
                                    tv = t.rearrange("p (a b) -> p a b", b=W)
                                    ve.tensor_tensor(
                                        out=y[:, r0:r1, :], in0=tv,
                                        in1=y[:, r0:r1, :], op=ALU.add)

                def conv_pool(ys, taps, bias, r0, r1):
                    # real TRN2 Pool has no scalar_tensor_tensor: plain
                    # tensor_scalar mult into a temp + tensor_tensor add
                    rows = r1 - r0
                    for p in range(CP):
                        y = ys[p]
                        x0v = x0_t[p][:, 0:PC * PC].rearrange("p (a b) -> p a b", b=PC)
                        first = True
                        for dy in range(3):
                            for dx in range(3):
                                i = dy * 3 + dx
                                src = x0v[:, r0 + dy:r1 + dy, dx:dx + W]
                                if first:
                                    nc.gpsimd.tensor_scalar(
                                        out=y[:, r0:r1, :], in0=src,
                                        scalar1=taps[p][:, i:i + 1],
                                        scalar2=bias[p],
                                        op0=ALU.mult, op1=ALU.add)
                                    first = False
                                else:
                                    t = ysb.tile([128, rows, W], F16, tag="tpool",
                                                 name="tpool", bufs=2,
                                                 padded_shape=[128, 16, W])
                                    nc.gpsimd.tensor_scalar(
                                        out=t, in0=src,
                                        scalar1=taps[p][:, i:i + 1],
                                        scalar2=None, op0=ALU.mult)
                                    nc.gpsimd.tensor_tensor(
                                        out=y[:, r0:r1, :], in0=t,
                                        in1=y[:, r0:r1, :], op=ALU.add)

                def conv_pe(dg_t, ys, bias, per0):
                    # rows [per0, H) via PE: per tap a diag(tap) matmul on the
                    # padded flat x stream, 9 taps accumulated in fp32 PSUM;
                    # bias + fp16 downcast ride the ACT eviction (junk pad
                    # columns never leave PSUM).
                    if per0 >= H:
                        return
                    rgs = [(r, min(r + 7, H)) for r in range(per0, H, 7)]
                    for p in range(CP):
                        base = p * 9
                        for rg0, rg1 in rgs:
                            cols = (rg1 - rg0) * PC
                            ps = mm_ps.tile([128, cols], F32, tag="mm",
                                            name="mm", padded_shape=[128, CHW])
                            for i in range(9):
                                dy, dx = i // 3, i % 3
                                off = (rg0 + dy) * PC + dx
                                nc.tensor.matmul(
                                    out=ps[:, :], lhsT=dg_t[:, base + i, :],
                                    rhs=x0_t[p][:, off:off + cols],
                                    start=(i == 0), stop=(i == 8))
                            psv = ps.rearrange("p (a b) -> p a b", b=PC)
                            nc.scalar.activation(
                                out=ys[p][:, rg0:rg1, :], in_=psv[:, :, 0:W],
                                func=ACTF.Identity, bias=bias[p], scale=1.0)

                yq_t = [ysb.tile([128, H, W], F16, tag=f"yq{p}", name=f"yq{p}")
                        for p in range(CP)]
                ykv_t = [ysb.tile([128, H, W], F16, tag=f"ykv{p}", name=f"ykv{p}")
                         for p in range(CP)]
                y_kv = [y.rearrange("p a b -> p (a b)") for y in ykv_t]
                y_q = [y.rearrange("p a b -> p (a b)") for y in yq_t]

                def y_chunk(y, ch):
                    return y[:, ch * CHW:(ch + 1) * CHW]

                # ---- persistent layout-A tiles ----
                # kA/vA carry 64 zero cols of slack: the XBAR DMA-transposes
                # read one 512-col window per chunk (448 + 64 overlap; the
                # last window's tail is the zero slack, contributing 0).
                vA = [big.tile([128, N + 64], F16, tag=f"vA{m}", name=f"vA{m}") for m in range(HP)]
                kA = [big.tile([128, N + 64], F16, tag=f"kA{m}", name=f"kA{m}") for m in range(HP)]
                qA = [big.tile([128, N], F16, tag=f"qA{m}", name=f"qA{m}") for m in range(HP)]
                for m in range(HP):
                    nc.vector.memset(vA[m][:, N:N + 64], 0.0)
                    nc.vector.memset(kA[m][:, N:N + 64], 0.0)

                # ---- front loop A: v,k projections + B-layout transposes +
                # direct PSUM ktv accumulation ----
                ktv_acc = ktv_ps.tile([128, HP, 128], F32, tag="ktva", name="ktva")

                def emit_tr(ch):
                    # XBAR DMA transpose straight SBUF->SBUF: no PE
                    # transposes, no PSUM staging, no ACT eviction. One
                    # 512-col window per (tensor, m) covers the 448-chunk;
                    # the 64-col overlap into the next chunk lands in block
                    # 3's partition rows 64:128, which the sliced matmuls
                    # skip (and the final window's tail is zero slack).
                    # Runs one chunk behind the evictions so the overlap
                    # read is of already-written data.
                    cols = slice(ch * CHW, ch * CHW + 512)
                    kT = [ev.tile([128, 4, 128], F16, tag=f"ktb{m}",
                                  name=f"ktb{m}", bufs=1) for m in range(HP)]
                    vT = [ev.tile([128, 4, 128], F16, tag=f"vtb{m}",
                                  name=f"vtb{m}", bufs=1) for m in range(HP)]
                    for m in range(HP):
                        nc.sync.dma_start_transpose(out=kT[m], in_=kA[m][:, cols])
                        nc.sync.dma_start_transpose(out=vT[m], in_=vA[m][:, cols])
                    for b in range(4):
                        rhi = 128 if b < 3 else 64
                        for mp in range(HP):
                            # start zeroes the whole 2KB bank, so only the
                            # very first matmul of the 4 interleaved chains
                            # may set it (and only the very last sets stop)
                            nc.tensor.matmul(
                                out=ktv_acc[:, mp, :],
                                lhsT=kT[mp][0:rhi, b, :],
                                rhs=vT[mp][0:rhi, b, :],
                                start=(ch == 0 and b == 0 and mp == 0),
                                stop=(ch == NCH - 1 and b == 3 and mp == HP - 1))

                def emit_A(ch):
                    for m in range(HP):
                        ps = mm_ps.tile([128, CHW], F32, tag="mm", name="mm")
                        for p in range(CP):
                            nc.tensor.matmul(
                                out=ps[:, :],
                                lhsT=wkv_t[p][:, INNER + m * 128:INNER + (m + 1) * 128],
                                rhs=y_chunk(y_kv[p], ch),
                                start=(p == 0), stop=(p == CP - 1))
                        nc.scalar.copy(out=ch_cols(vA[m], ch), in_=ps[:, :])
                    for m in range(HP):
                        ps = mm_ps.tile([128, CHW], F32, tag="mm", name="mm")
                        for p in range(CP):
                            nc.tensor.matmul(
                                out=ps[:, :],
                                lhsT=wkv_t[p][:, m * 128:(m + 1) * 128],
                                rhs=y_chunk(y_kv[p], ch),
                                start=(p == 0), stop=(p == CP - 1))
                        nc.scalar.copy(out=ch_cols(kA[m], ch), in_=ps[:, :])
                    if ch > 0:
                        emit_tr(ch - 1)

                # ---- front loop B: q projection + diag broadcast ----
                # Per chunk: stream the 4 m-tiles' q projections on PE with
                # no interleaved dependents (qk reads the PSUM directly on
                # DVE; qA eviction on ACT runs in parallel), then emit the
                # previous chunk's sb matmuls (lhsT = block-diag J64 turns
                # the qk stream directly into per-head-broadcast diag).
                sb16 = [None] * HP

                def make_sb16():
                    # created AFTER the dg_q band so the "dg" tag reuse
                    # sequence is dg_k -> dg_q -> sb3
                    for m in range(HP):
                        sb16[m] = (xsb.tile([128, N], F16, tag=f"x0{m}",
                                            name=f"sb{m}")
                                   if m < CP else
                                   wsb.tile([128, N], F16, tag="dg",
                                            name="sb3"))
                # per-(m, chunk) row-sum partials of sb (free via accum_out on
                # the evictions) -> m0 without any standalone big reduce
                parts = [small.tile([128, 8], F32, tag=f"part{m}", name=f"part{m}",
                                    bufs=1) for m in range(HP)]
                for m in range(HP):
                    nc.vector.memset(parts[m], 0.0)

                def emit_sb(ch, qks):
                    for m in range(HP):
                        sb_ps = kv_ps.tile([128, CHW], F32, tag="sbps",
                                           name="sbps")
                        nc.tensor.matmul(out=sb_ps[:, :], lhsT=hsel2_t,
                                         rhs=qks[m], start=True, stop=True)
                        nc.scalar.activation(
                            out=ch_cols(sb16[m], ch), in_=sb_ps[:, :],
                            func=ACTF.Copy,
                            accum_out=parts[m][:, ch:ch + 1])

                _prev_qk = [None]
                oach_all = {}

                # ---- ktv block-diagonal + oa stage (runs inside loop B) ----
                ktv_bd_h = [None]

                def emit_ktv_bd():
                    ktv_bd = small.tile([128, HP, 128], F16, tag="ktvbd",
                                        name="ktvbd")
                    nc.vector.memset(ktv_bd, 0.0)
                    nc.scalar.copy(out=ktv_bd[0:DIM_HEAD, :, 0:DIM_HEAD],
                                   in_=ktv_acc[0:DIM_HEAD, :, 0:DIM_HEAD])
                    nc.scalar.copy(out=ktv_bd[DIM_HEAD:128, :, DIM_HEAD:128],
                                   in_=ktv_acc[DIM_HEAD:128, :, DIM_HEAD:128])
                    ktv_bd_h[0] = ktv_bd

                def emit_oa(ch):
                    oach = []
                    for m in range(HP):
                        ew = ev.tile([128, CHW], F16, tag="ew", name="ew", bufs=2)
                        nc.gpsimd.tensor_tensor(out=ew, in0=ch_cols(sb16[m], ch),
                                                in1=ch_cols(vA[m], ch), op=ALU.mult)
                        oa_ps = mm_ps.tile([128, CHW], F32, tag="mm", name="mm")
                        nc.tensor.matmul(out=oa_ps[:, :], lhsT=ktv_bd_h[0][:, m, :],
                                         rhs=ch_cols(qA[m], ch),
                                         start=True, stop=True)
                        oa = ev.tile([128, CHW], F16, tag=f"oa{m}", name=f"oa{m}",
                                     bufs=3)
                        # must be DVE: GPSIMD/Pool cannot read PSUM, and
                        # ACT has no two-tensor op
                        nc.vector.tensor_tensor(out=oa, in0=oa_ps[:, :], in1=ew,
                                                op=ALU.subtract)
                        oach.append(oa)
                    return oach

                def emit_B(ch):
                    prev_qk = _prev_qk[0]
                    cur_qk = []
                    for m in range(HP):
                        ps = mm_ps.tile([128, CHW], F32, tag="mm", name="mm")
                        for p in range(CP):
                            nc.tensor.matmul(
                                out=ps[:, :],
                                lhsT=wq_t[p][:, m * 128:(m + 1) * 128],
                                rhs=y_chunk(y_q[p], ch),
                                start=(p == 0), stop=(p == CP - 1))
                        nc.scalar.copy(out=ch_cols(qA[m], ch), in_=ps[:, :])
                        qk = ev.tile([128, CHW], F16, tag=f"qk{m}",
                                     name=f"qk{m}", bufs=2)
                        qeng = nc.gpsimd if m % 2 == 0 else nc.vector
                        qeng.tensor_tensor(out=qk, in0=ch_cols(qA[m], ch),
                                           in1=ch_cols(kA[m], ch), op=ALU.mult)
                        cur_qk.append(qk)
                    if prev_qk is not None:
                        emit_sb(ch - 1, prev_qk)
                    _prev_qk[0] = cur_qk

                # Ladder sizing: only the FIRST kv group is small (to unblock
                # loop A early); after that the makespan is set by when each
                # group's LAST rows land, so bigger groups (less per-op
                # overhead) win.
                def bands(a, b, step=11):
                    if b <= a:
                        return []
                    n = max(1, round((b - a) / step))
                    edges = [a + (b - a) * i // n for i in range(n + 1)]
                    return list(zip(edges[:-1], edges[1:]))
                conv_pe(dg_k, ykv_t, bk_t, per0k)
                kb = bands(kd, per0k, cfg["pstep"])
                qb = bands(qd, per0q, cfg["pstep"])
                # Pool's early idle goes to the q MIDDLE band (q has no
                # kv dependency), so DVE's q share -- which gates loop B --
                # shrinks to the first rows only.
                conv_dve(ykv_t, tk_t, bk_t, 0, 8)
                for g in kb:
                    conv_pool(ykv_t, tk_t, bk_t, *g)
                conv_dve(ykv_t, tk_t, bk_t, 8, 24)
                conv_dve(ykv_t, tk_t, bk_t, 24, kd)
                for g in qb:
                    conv_pool(yq_t, tq_t, bq_t, *g)
                conv_dve(yq_t, tq_t, bq_t, 0, 24)
                if qd > 24:
                    conv_dve(yq_t, tq_t, bq_t, 24, qd)
                for ch in range(NCH):
                    emit_A(ch)
                emit_tr(NCH - 1)
                emit_ktv_bd()
                dg_q = wsb.tile([128, 27, 128], F16, tag="dg", name="dgq")
                nc.sync.dma_start(out=dg_q, in_=dg_d[:, 27 * 128:])
                conv_pe(dg_q, yq_t, bq_t, per0q)
                make_sb16()
                for ch in range(NCH):
                    emit_B(ch)
                emit_sb(NCH - 1, _prev_qk[0])

                # ---- m0: global scalar via 1-element AllReduce ----
                # m0 = sum(diag) = sum(sb16)/64 (every head row repeats 64x)
                # Emitted mid-back-half (after oa(1)) so the collective's
                # latency doesn't block Pool's first oa evictions; final(0)
                # only touches m0I in its last accumulation matmul.
                m0I_h = [None]

                def emit_m0():
                    m0_ps = kv_ps.tile([1, 8], F32, tag="sbps", name="m0ps")
                    for m in range(HP):
                        nc.tensor.matmul(out=m0_ps[:, :], lhsT=o64_t, rhs=parts[m],
                                         start=(m == 0), stop=(m == HP - 1))
                    m0s = small.tile([1, 1], F32, tag="m0s", name="m0s")
                    nc.vector.tensor_reduce(out=m0s, in_=m0_ps[:, :],
                                            axis=mybir.AxisListType.X, op=ALU.add)
                    cc = dram.tile([1, 1], F32, tag="cc", name="cc")
                    nc.gpsimd.dma_start(out=cc[:, :], in_=m0s)
                    if not no_cc:
                        nc.gpsimd.collective_compute(
                            "AllReduce", ALU.add, replica_groups=[list(range(8))],
                            ins=[cc[:, :].opt()], outs=[cc[:, :].opt()])
                    m0b = small.tile([128, 1], F32, tag="m0b", name="m0b")
                    nc.gpsimd.dma_start(out=m0b, in_=cc[:, :].partition_broadcast(128))
                    # m0wo = m0 * Wo -- folds the m0*v term into the final
                    # PSUM accumulation as 4 extra (m0 Wo) @ v chains, so no
                    # separate Wo@v pass or its evictions are needed and the
                    # collective still overlaps the back half (final only
                    # touches m0wo in its second accumulation chain).
                    m0wo = [small.tile([128, DIM], F16, tag=f"m0wo{kt}",
                                       name=f"m0wo{kt}", bufs=1)
                            for kt in range(HP)]
                    for kt in range(HP):
                        nc.gpsimd.tensor_scalar(out=m0wo[kt], in0=wo_t[kt],
                                                scalar1=m0b, scalar2=None,
                                                op0=ALU.mult)
                    m0I_h[0] = m0wo

                # (ktv_bd / emit_oa are defined above, near the loop-B defs)

                def emit_final(ch, oach):
                    m0wo = m0I_h[0]
                    for ot in range(CP):
                        ps = kv_ps.tile([128, CHW], F32, tag="sbps", name="fps")
                        for kt in range(HP):
                            nc.tensor.matmul(
                                out=ps[:, :],
                                lhsT=wo_t[kt][:, ot * 128:(ot + 1) * 128],
                                rhs=oach[kt],
                                start=(kt == 0), stop=False)
                        for kt in range(HP):
                            nc.tensor.matmul(
                                out=ps[:, :],
                                lhsT=m0wo[kt][:, ot * 128:(ot + 1) * 128],
                                rhs=ch_cols(vA[kt], ch),
                                start=False, stop=(kt == HP - 1))
                        of = ev.tile([128, CHW], F32, tag="of", name="of", bufs=2)
                        nc.scalar.activation(out=of, in_=ps[:, :],
                                             func=ACTF.Identity,
                                             bias=bo_t[ot], scale=1.0)
                        nc.sync.dma_start(
                            out=out_d[ot * 128:(ot + 1) * 128,
                                      ch * CHW:(ch + 1) * CHW],
                            in_=of)

                pend = []
                for ch in range(NCH):
                    oach = emit_oa(ch)
                    pend.append((ch, oach))
                    if ch == 1:
                        emit_m0()
                    if len(pend) > 2:
                        e = pend.pop(0)
                        emit_final(e[0], e[1])
                for e in pend:
                    emit_final(e[0], e[1])
            if loop_n is None:
                for _ in range(reps):
                    emit_body()
            else:
                with tc.For_i(0, loop_n, 1):
                    emit_body()
    nc.finalize()
    return nc


def _get_nc(reps: int = 1, loop_n=None, no_cc=False):
    key = (reps, loop_n, no_cc)
    if key not in _CACHE:
        _CACHE[key] = _build(reps, loop_n, no_cc)
    return _CACHE[key]


def prepare_in_maps(inputs):
    """Host-side preprocessing: fold BN, pad/shift x, transpose weights."""
    x = np.asarray(inputs["x"], np.float32)

    def fold(dw, g, b, m, v):
        inv = np.asarray(g, np.float32) / np.sqrt(np.asarray(v, np.float32) + EPS)
        taps = np.asarray(dw, np.float32)[:, 0].reshape(DIM, 9) * inv[:, None]
        bias = np.asarray(b, np.float32) - np.asarray(m, np.float32) * inv
        return (np.ascontiguousarray(taps, np.float32),
                np.ascontiguousarray(bias[:, None], np.float32))

    tq, bq = fold(inputs["wq_dw"], inputs["wq_bn_g"], inputs["wq_bn_b"],
                  inputs["wq_bn_m"], inputs["wq_bn_v"])
    tk, bk = fold(inputs["wkv_dw"], inputs["wkv_bn_g"], inputs["wkv_bn_b"],
                  inputs["wkv_bn_m"], inputs["wkv_bn_v"])
    tb = np.ascontiguousarray(
        np.concatenate([tk, bk, tq, bq], axis=1), np.float32)
    dg = np.zeros((128, 54, 128), np.float16)
    for br, taps in enumerate((tk, tq)):
        for p in range(3):
            for i in range(9):
                j = (br * 3 + p) * 9 + i
                np.fill_diagonal(dg[:, j, :],
                                 taps[p * 128:(p + 1) * 128, i].astype(np.float16))
    dg = np.ascontiguousarray(dg.reshape(128, 54 * 128))
    wqT = _f16((SCALE * np.asarray(inputs["wq_pw"], np.float32)).T)
    wkvT = _f16(np.asarray(inputs["wkv_pw"], np.float32).T)
    woT = _f16(np.asarray(inputs["wo"], np.float32).T)
    bo = np.ascontiguousarray(np.asarray(inputs["bo"], np.float32)[:, None])
    hsel2 = np.zeros((128, 128), np.float32)
    hsel2[:64, :64] = 1.0
    hsel2[64:, 64:] = 1.0
    hsel2 = _f16(hsel2)
    posI = _f16(np.eye(128, dtype=np.float32))
    o64 = np.ascontiguousarray(np.full((128, 1), 1.0 / DIM_HEAD, np.float32))

    xpad = np.zeros((B, DIM, PC, PC), np.float16)
    xpad[:, :, 1:1 + H, 1:1 + W] = x.astype(np.float16)
    xflat = np.zeros((B, DIM, XL), np.float16)
    xflat[:, :, :PC * PC] = xpad.reshape(B, DIM, PC * PC)
    xsh = np.zeros_like(xflat)
    xsh[:, :, :XL - 1] = xflat[:, :, 1:]
    shared = dict(tb=tb, dg=dg, wqT=wqT, wkvT=wkvT, woT=woT,
                  bo=bo, hsel2=hsel2, posI=posI, o64=o64)
    return [dict(shared, xp=np.ascontiguousarray(xflat[b])) for b in range(B)]


def kernel(**inputs) -> np.ndarray:
    from concourse.bass_utils import run_bass_kernel_spmd
    in_maps = prepare_in_maps(inputs)
    nc = _get_nc(1)
    res = run_bass_kernel_spmd(nc, in_maps, list(range(8)))
    out = np.stack([res.results[b]["out"] for b in range(B)])
    return np.ascontiguousarray(out.reshape(B, DIM, H, W).astype(np.float32))

